# revision 5
# baseline (speedup 1.0000x reference)
"""BiLSTM (embedding -> fwd/bwd LSTM last-state -> dense -> sigmoid) on 8 trn2 cores.

Data-parallel: 256 batch rows per core. On-chip layout is fully transposed
(units/gates on partitions, batch on the free dim) so the recurrence needs no
per-step transposes:
  - z^T gate tiles [128, 2048] f32 in PSUM per direction (cols: i|f|o|g, each
    512 = units-lo 256 batch | units-hi 256 batch)
  - x_t^T via dma_gather(transpose=True) straight from the bf16 embedding table
  - h^T bf16 [128, 512], c^T/O^T f32 [128, 512] persistent per direction
  - keras mask_zero semantics via copy_predicated with a broadcast mask tile
Both directions are interleaved in one static loop so PE/ACT/DVE overlap.
"""

import numpy as np
import ml_dtypes

import concourse.bass as bass
import concourse.bacc as bacc
import concourse.mybir as mybir
from concourse import tile
from concourse.bass_utils import run_bass_kernel_spmd

VOCAB, EMB, UNITS, B, T = 10000, 128, 256, 2048, 256
NCORES = 8
BC = B // NCORES          # 256 batch rows per core
DT_BF = mybir.dt.bfloat16
DT_F32 = mybir.dt.float32
DT_I16 = mybir.dt.int16
AF = mybir.ActivationFunctionType
OP = mybir.AluOpType

# PSUM z^T column layout: [i | f | o | g], each 512 cols (units-lo|units-hi).
# chunk c of W/U (units c*128:(c+1)*128 of the 1024 gate dim) -> psum col slot:
# i: chunks 0,1  f: 2,3  g: 4,5  o: 6,7   (keras order i,f,g,o)
PSUM_CHUNKS = [0, 1, 2, 3, 6, 7, 4, 5]   # psum slot s <- W chunk PSUM_CHUNKS[s]

TRACE = False          # test.py flips this for profiled runs
TRACE_KWARGS = {}
T_STEPS = T            # test overrides for small-kernel bring-up
LAST_RESULTS = None    # stash for test.py (exec_time etc.)


def _bf16(x):
    return np.asarray(x, dtype=ml_dtypes.bfloat16)


def build(nc, Tst, zero_gate_bias, b2_val):
    embD = nc.dram_tensor("emb_bf", [VOCAB, EMB], DT_BF, kind="ExternalInput")
    idxD = nc.dram_tensor("idx16", [128, Tst * 16], DT_I16, kind="ExternalInput")
    maskD = nc.dram_tensor("maskD", [Tst, 2 * BC], DT_I16, kind="ExternalInput")
    wD = {}
    for d in ("f", "b"):
        wD[f"W{d}"] = nc.dram_tensor(f"W{d}_sb", [128, 1024], DT_BF, kind="ExternalInput")
        wD[f"U{d}"] = nc.dram_tensor(f"U{d}_sb", [128, 2048], DT_BF, kind="ExternalInput")
    biasD = nc.dram_tensor("bias_sb", [128, 16], DT_F32, kind="ExternalInput")
    w1D = nc.dram_tensor("W1_sb", [128, 512], DT_BF, kind="ExternalInput")
    b1D = nc.dram_tensor("b1_sb", [128, 1], DT_F32, kind="ExternalInput")
    w2D = nc.dram_tensor("W2_sb", [128, 1], DT_BF, kind="ExternalInput")
    outD = nc.dram_tensor("out", [1, BC], DT_F32, kind="ExternalOutput")

    with tile.TileContext(nc) as tc:
        with (
            tc.tile_pool(name="const", bufs=1) as cpool,
            tc.tile_pool(name="state", bufs=1) as spool,
            tc.tile_pool(name="work", bufs=2) as wpool,
            tc.tile_pool(name="xin", bufs=4) as xpool,
            tc.tile_pool(name="msk", bufs=3) as mpool,
            tc.tile_pool(name="psum", bufs=1, space="PSUM") as ppool,
        ):
            # ---- constants -------------------------------------------------
            W = {}
            for d in ("f", "b"):
                W[f"W{d}"] = cpool.tile([128, 1024], DT_BF, tag=f"W{d}", name=f"W{d}")
                nc.sync.dma_start(W[f"W{d}"][:], wD[f"W{d}"][:])
                W[f"U{d}"] = cpool.tile([128, 2048], DT_BF, tag=f"U{d}", name=f"U{d}")
                nc.sync.dma_start(W[f"U{d}"][:], wD[f"U{d}"][:])
            bias = cpool.tile([128, 16], DT_F32, tag="bias", name="bias")
            nc.sync.dma_start(bias[:], biasD[:])
            W1 = cpool.tile([128, 512], DT_BF, tag="W1", name="W1")
            nc.sync.dma_start(W1[:], w1D[:])
            b1 = cpool.tile([128, 1], DT_F32, tag="b1", name="b1")
            nc.sync.dma_start(b1[:], b1D[:])
            W2 = cpool.tile([128, 1], DT_BF, tag="W2", name="W2")
            nc.sync.dma_start(W2[:], w2D[:])
            idx = cpool.tile([128, Tst * 16], DT_I16, tag="idx", name="idx")
            nc.sync.dma_start(idx[:], idxD[:])

            # ---- persistent state ------------------------------------------
            st = {}
            for d in ("f", "b"):
                st[f"h{d}"] = spool.tile([128, 512], DT_BF, tag=f"h{d}", name=f"h{d}")
                st[f"c{d}"] = spool.tile([128, 512], DT_F32, tag=f"c{d}", name=f"c{d}")
                st[f"o{d}"] = spool.tile([128, 512], DT_F32, tag=f"o{d}", name=f"o{d}")
                nc.vector.memset(st[f"h{d}"][:], 0.0)
                nc.vector.memset(st[f"c{d}"][:], 0.0)
                nc.vector.memset(st[f"o{d}"][:], 0.0)
            zP = {d: ppool.tile([128, 2048], DT_F32, tag=f"z{d}", name=f"z{d}") for d in ("f", "b")}

            # ---- recurrence ------------------------------------------------
            def step(d, t, qn):
                h, c, o = st[f"h{d}"], st[f"c{d}"], st[f"o{d}"]
                z = zP[d]
                xt = xpool.tile([128, 1, BC], DT_BF, tag=f"x{d}", name=f"x{d}")
                nc.gpsimd.dma_gather(
                    xt[:], embD[:], idx[:, t * 16:(t + 1) * 16],
                    num_idxs=BC, num_idxs_reg=BC, elem_size=EMB,
                    transpose=True, queue_num=0,
                )
                msk = mpool.tile([128, 2 * BC], DT_I16, tag=f"m{d}", name=f"m{d}")
                nc.sync.dma_start(msk[:], maskD[t].partition_broadcast(128))

                for s in range(8):
                    cch = PSUM_CHUNKS[s]
                    col = s * BC
                    out_ap = z[:, col:col + BC]
                    nc.tensor.matmul(out_ap, W[f"W{d}"][:, cch * 128:(cch + 1) * 128],
                                     xt[:, 0, :], start=True, stop=False)
                    nc.tensor.matmul(out_ap, W[f"U{d}"][:, (cch * 2) * 128:(cch * 2 + 1) * 128],
                                     h[:, 0:BC], start=False, stop=False)
                    nc.tensor.matmul(out_ap, W[f"U{d}"][:, (cch * 2 + 1) * 128:(cch * 2 + 2) * 128],
                                     h[:, BC:2 * BC], start=False, stop=True)

                sig = wpool.tile([128, 1536], DT_F32, tag=f"sig{d}", name=f"sig{d}")
                gac = wpool.tile([128, 512], DT_F32, tag=f"g{d}", name=f"g{d}")
                if zero_gate_bias:
                    nc.scalar.activation(sig[:], z[:, 0:1536], AF.Sigmoid)
                    nc.scalar.activation(gac[:], z[:, 1536:2048], AF.Tanh)
                else:
                    boff = 0 if d == "f" else 8
                    for s in range(8):
                        fn = AF.Sigmoid if s < 6 else AF.Tanh
                        dst = sig[:, s * BC:(s + 1) * BC] if s < 6 else gac[:, (s - 6) * BC:(s - 5) * BC]
                        nc.scalar.activation(dst, z[:, s * BC:(s + 1) * BC], fn,
                                             bias=bias[:, boff + s:boff + s + 1])

                ig = wpool.tile([128, 512], DT_F32, tag=f"ig{d}", name=f"ig{d}")
                nc.vector.tensor_tensor(ig[:], sig[:, 0:512], gac[:], op=OP.mult)
                fc = wpool.tile([128, 512], DT_F32, tag=f"fc{d}", name=f"fc{d}")
                nc.vector.tensor_tensor(fc[:], sig[:, 512:1024], c[:], op=OP.mult)
                cn = wpool.tile([128, 512], DT_F32, tag=f"cn{d}", name=f"cn{d}")
                nc.vector.tensor_tensor(cn[:], ig[:], fc[:], op=OP.add)
                nc.vector.copy_predicated(c[:], msk[:], cn[:])
                nc.vector.copy_predicated(o[:], msk[:], sig[:, 1024:1536])
                th = wpool.tile([128, 512], DT_F32, tag=f"th{d}", name=f"th{d}")
                nc.scalar.activation(th[:], c[:], AF.Tanh)
                nc.vector.tensor_tensor(h[:], o[:], th[:], op=OP.mult)

            for t in range(Tst):
                step("f", t, 0)
                step("b", Tst - 1 - t, 1)

            # ---- dense head ------------------------------------------------
            dP = ppool.tile([128, BC], DT_F32, tag="zf", name="zf")
            hs = [st["hf"][:, 0:BC], st["hf"][:, BC:2 * BC],
                  st["hb"][:, 0:BC], st["hb"][:, BC:2 * BC]]
            for k in range(4):
                nc.tensor.matmul(dP[:, :], W1[:, k * 128:(k + 1) * 128], hs[k],
                                 start=(k == 0), stop=(k == 3))
            dT = wpool.tile([128, BC], DT_BF, tag="dT", name="dT")
            nc.scalar.activation(dT[:], dP[:, :], AF.Relu, bias=b1[:])
            oP = ppool.tile([1, BC], DT_F32, tag="zb", name="zb")
            nc.tensor.matmul(oP[:, :], W2[:], dT[:], start=True, stop=True)
            oS = wpool.tile([1, BC], DT_F32, tag="oS", name="oS")
            nc.scalar.activation(oS[:], oP[:, :], AF.Sigmoid, bias=float(b2_val))
            nc.sync.dma_start(outD[:], oS[:])
    return nc


def prep_core_inputs(tok_c, emb, Wf, Uf, bf, Wb, Ub, bb, W1, b1, W2, b2, Tst):
    """tok_c: [BC, T] int array for this core -> in_map dict."""
    tokT = np.ascontiguousarray(tok_c.T)[:Tst]          # [Tst, BC]
    r = tokT.astype(np.int16).reshape(Tst, BC // 16, 16)
    # wrapped in 16 partitions AND replicated to all 8 Q7 cores (16 each)
    idx16 = np.tile(r.transpose(2, 0, 1).reshape(16, Tst * 16), (8, 1))
    m = (tokT != 0).astype(np.int16)                    # [Tst, BC]
    maskD = np.concatenate([m, m], axis=1)              # [Tst, 2*BC]

    def u_sb(U):
        return _bf16(U.reshape(2, 128, 8, 128).transpose(1, 2, 0, 3).reshape(128, 2048))

    return {
        "emb_bf": _bf16(emb),
        "idx16": idx16,
        "maskD": maskD,
        "Wf_sb": _bf16(Wf), "Wb_sb": _bf16(Wb),
        "Uf_sb": u_sb(Uf), "Ub_sb": u_sb(Ub),
        "bias_sb": np.concatenate(
            [b.reshape(8, 128)[PSUM_CHUNKS].T.astype(np.float32) for b in (bf, bb)],
            axis=1),
        "W1_sb": _bf16(W1.reshape(4, 128, 128).transpose(1, 0, 2).reshape(128, 512)),
        "b1_sb": b1.astype(np.float32).reshape(128, 1),
        "W2_sb": _bf16(W2.reshape(128, 1)),
    }


def kernel(tokens, emb, Wf, Uf, bf, Wb, Ub, bb, W1, b1, W2, b2):
    global LAST_RESULTS
    tokens = np.asarray(tokens)
    emb, Wf, Uf, bf = map(np.asarray, (emb, Wf, Uf, bf))
    Wb, Ub, bb = map(np.asarray, (Wb, Ub, bb))
    W1, b1, W2, b2 = map(np.asarray, (W1, b1, W2, b2))
    Tst = T_STEPS
    zero_gate_bias = not (np.any(bf[: 4 * UNITS]) or np.any(bb[: 4 * UNITS]))

    nc = bacc.Bacc("TRN2", target_bir_lowering=False, debug=False,
                   num_devices=NCORES)
    build(nc, Tst, zero_gate_bias, float(b2.reshape(-1)[0]))
    nc.compile()

    in_maps = []
    for c in range(NCORES):
        tok_c = tokens[c * BC:(c + 1) * BC]
        in_maps.append(prep_core_inputs(tok_c, emb, Wf, Uf, bf, Wb, Ub, bb,
                                        W1, b1, W2, b2, Tst))
    res = run_bass_kernel_spmd(nc, in_maps, core_ids=list(range(NCORES)),
                               trace=TRACE, trace_kwargs=TRACE_KWARGS)
    LAST_RESULTS = res
    out = np.concatenate([np.asarray(r["out"]).reshape(BC) for r in res.results])
    return out.reshape(B, 1).astype(np.float32)
